# revision 1
# baseline (speedup 1.0000x reference)
import numpy as np
import jax
import jax.numpy as jnp
from jax import lax

# Hardcoded problem shapes (nn_GCNTransformerClassifier)
N, B, NPG = 16384, 32, 512
E = 262144
F_IN, D, H, DH, FF = 2048, 256, 8, 32, 1024
L_GCN, L_TF, NCLS = 3, 2, 2
EPS_BN = 1e-5
EPS_LN = 1e-5
M = 8                      # cores
NPD = N // M               # nodes per device (2048 = 4 graphs)
BPD = B // M               # graphs per device
S = NPG + 1


def _layernorm(x, g, b):
    m = jnp.mean(x, axis=-1, keepdims=True)
    v = jnp.mean(jnp.square(x - m), axis=-1, keepdims=True)
    return (x - m) * lax.rsqrt(v + EPS_LN) * g + b


def _tf_and_head(hg, p):
    # hg: [NPD, D] nodes of this device's BPD graphs
    seq = hg.reshape(BPD, NPG, D)
    cls = jnp.broadcast_to(p["cls"], (BPD, 1, D))
    seq = jnp.concatenate([cls, seq], axis=1)  # [BPD, S, D]
    scale = 1.0 / np.sqrt(DH)
    for l in range(L_TF):
        q = (seq @ p["Wq"][l] + p["bq"][l]).reshape(BPD, S, H, DH)
        k = (seq @ p["Wk"][l] + p["bk"][l]).reshape(BPD, S, H, DH)
        v = (seq @ p["Wv"][l] + p["bv"][l]).reshape(BPD, S, H, DH)
        att = jax.nn.softmax(jnp.einsum("bqhd,bkhd->bhqk", q, k) * scale, axis=-1)
        o = jnp.einsum("bhqk,bkhd->bqhd", att, v).reshape(BPD, S, D)
        seq = _layernorm(seq + o @ p["Wo"][l] + p["bo"][l], p["ln1_g"][l], p["ln1_b"][l])
        ff = jax.nn.gelu(seq @ p["ff_W1"][l] + p["ff_b1"][l], approximate=False) @ p["ff_W2"][l] + p["ff_b2"][l]
        seq = _layernorm(seq + ff, p["ln2_g"][l], p["ln2_b"][l])
    feat = seq[:, 0, :]
    hcl = jax.nn.relu(feat @ p["cl_W1"] + p["cl_b1"])
    return hcl @ p["cl_W2"] + p["cl_b2"]  # [BPD, NCLS]


def _gcn_layers(h, src, dst, norm, selfnorm, p):
    # h: full [N, D]; scatter over all nodes (replicated on each device)
    for i in range(L_GCN):
        res = h
        hw = h @ p["gcn_W"][i]
        m = hw[src] * norm[:, None]
        agg = jax.ops.segment_sum(m, dst, num_segments=N) + selfnorm[:, None] * hw
        h = agg + p["gcn_b"][i]
        h = (h - p["bn_m"][i]) * lax.rsqrt(p["bn_v"][i] + EPS_BN) * p["bn_g"][i] + p["bn_b"][i]
        h = jax.nn.relu(h)
        if i > 0:
            h = h + res
    return h


def _fn_allgather(x_d, src, dst, norm, selfnorm, p):
    h_d = jax.nn.relu(x_d @ p["in_W"] + p["in_b"])          # [NPD, D]
    h = lax.all_gather(h_d, "i", tiled=True)                 # [N, D]
    h = _gcn_layers(h, src, dst, norm, selfnorm, p)
    d = lax.axis_index("i")
    hg = lax.dynamic_slice_in_dim(h, d * NPD, NPD, axis=0)
    return _tf_and_head(hg, p)


def _fn_replicated(x, src, dst, norm, selfnorm, p):
    h = jax.nn.relu(x @ p["in_W"] + p["in_b"])               # [N, D] replicated
    h = _gcn_layers(h, src, dst, norm, selfnorm, p)
    d = lax.axis_index("i")
    hg = lax.dynamic_slice_in_dim(h, d * NPD, NPD, axis=0)
    return _tf_and_head(hg, p)


def _prep(edge_index):
    src = edge_index[0].astype(np.int32)
    dst = edge_index[1].astype(np.int32)
    deg = np.bincount(dst, minlength=N).astype(np.float32) + 1.0
    dinv = 1.0 / np.sqrt(np.maximum(deg, 1.0))
    norm = (dinv[src] * dinv[dst]).astype(np.float32)
    selfnorm = (dinv * dinv).astype(np.float32)
    return src, dst, norm, selfnorm


def _run_neuron(x, edge_index, params):
    devs = jax.devices()
    if len(devs) < M:
        raise RuntimeError("need 8 devices")
    src, dst, norm, selfnorm = _prep(edge_index)
    p = {k: jnp.asarray(v) for k, v in params.items()}
    xs = np.asarray(x).reshape(M, NPD, F_IN)
    try:
        f = jax.pmap(_fn_allgather, axis_name="i",
                     in_axes=(0, None, None, None, None, None),
                     devices=devs[:M])
        out = f(xs, jnp.asarray(src), jnp.asarray(dst), jnp.asarray(norm),
                jnp.asarray(selfnorm), p)
        return np.asarray(out).reshape(B, NCLS)
    except Exception:
        f = jax.pmap(_fn_replicated, axis_name="i",
                     in_axes=(None, None, None, None, None, None),
                     devices=devs[:M])
        out = f(jnp.asarray(np.asarray(x)), jnp.asarray(src), jnp.asarray(dst),
                jnp.asarray(norm), jnp.asarray(selfnorm), p)
        return np.asarray(out).reshape(B, NCLS)


def _run_cpu(x, edge_index, batch, params):
    cpu = jax.devices("cpu")[0]
    with jax.default_device(cpu):
        src, dst, norm, selfnorm = _prep(edge_index)
        p = {k: jnp.asarray(np.asarray(v)) for k, v in params.items()}
        h = jax.nn.relu(jnp.asarray(np.asarray(x)) @ p["in_W"] + p["in_b"])
        h = _gcn_layers(h, jnp.asarray(src), jnp.asarray(dst),
                        jnp.asarray(norm), jnp.asarray(selfnorm), p)
        outs = []
        for d in range(M):
            outs.append(_tf_and_head(h[d * NPD:(d + 1) * NPD], p))
        return np.concatenate([np.asarray(o) for o in outs], axis=0)


def kernel(x, edge_index, batch, params):
    try:
        return _run_neuron(x, edge_index, params)
    except Exception:
        return _run_cpu(x, edge_index, batch, params)


# revision 2
# speedup vs baseline: 1.1604x; 1.1604x over previous
import numpy as np
import jax
import jax.numpy as jnp
from jax import lax

# Hardcoded problem shapes (nn_GCNTransformerClassifier)
N, B, NPG = 16384, 32, 512
E = 262144
F_IN, D, H, DH, FF = 2048, 256, 8, 32, 1024
L_GCN, L_TF, NCLS = 3, 2, 2
EPS_BN = 1e-5
EPS_LN = 1e-5
M = 8                      # cores
NPD = N // M               # nodes per device (2048 = 4 graphs)
BPD = B // M               # graphs per device
S = NPG + 1


def _layernorm(x, g, b):
    m = jnp.mean(x, axis=-1, keepdims=True)
    v = jnp.mean(jnp.square(x - m), axis=-1, keepdims=True)
    return (x - m) * lax.rsqrt(v + EPS_LN) * g + b


def _tf_and_head(hg, p):
    # hg: [NPD, D] nodes of this device's BPD graphs
    seq = hg.reshape(BPD, NPG, D)
    cls = jnp.broadcast_to(p["cls"], (BPD, 1, D))
    seq = jnp.concatenate([cls, seq], axis=1)  # [BPD, S, D]
    scale = 1.0 / np.sqrt(DH)
    for l in range(L_TF):
        q = (seq @ p["Wq"][l] + p["bq"][l]).reshape(BPD, S, H, DH)
        k = (seq @ p["Wk"][l] + p["bk"][l]).reshape(BPD, S, H, DH)
        v = (seq @ p["Wv"][l] + p["bv"][l]).reshape(BPD, S, H, DH)
        att = jax.nn.softmax(jnp.einsum("bqhd,bkhd->bhqk", q, k) * scale, axis=-1)
        o = jnp.einsum("bhqk,bkhd->bqhd", att, v).reshape(BPD, S, D)
        seq = _layernorm(seq + o @ p["Wo"][l] + p["bo"][l], p["ln1_g"][l], p["ln1_b"][l])
        ff = jax.nn.gelu(seq @ p["ff_W1"][l] + p["ff_b1"][l], approximate=False) @ p["ff_W2"][l] + p["ff_b2"][l]
        seq = _layernorm(seq + ff, p["ln2_g"][l], p["ln2_b"][l])
    feat = seq[:, 0, :]
    hcl = jax.nn.relu(feat @ p["cl_W1"] + p["cl_b1"])
    return hcl @ p["cl_W2"] + p["cl_b2"]  # [BPD, NCLS]


def _gcn_layers(h, src, dst, norm, selfnorm, p):
    # h: full [N, D]; scatter over all nodes (replicated on each device)
    for i in range(L_GCN):
        res = h
        hw = h @ p["gcn_W"][i]
        m = hw[src] * norm[:, None]
        agg = jax.ops.segment_sum(m, dst, num_segments=N) + selfnorm[:, None] * hw
        h = agg + p["gcn_b"][i]
        h = (h - p["bn_m"][i]) * lax.rsqrt(p["bn_v"][i] + EPS_BN) * p["bn_g"][i] + p["bn_b"][i]
        h = jax.nn.relu(h)
        if i > 0:
            h = h + res
    return h


def _fn_allgather(x_d, src, dst, norm, selfnorm, p):
    h_d = jax.nn.relu(x_d @ p["in_W"] + p["in_b"])          # [NPD, D]
    h = lax.all_gather(h_d, "i", tiled=True)                 # [N, D]
    h = _gcn_layers(h, src, dst, norm, selfnorm, p)
    d = lax.axis_index("i")
    hg = lax.dynamic_slice_in_dim(h, d * NPD, NPD, axis=0)
    return _tf_and_head(hg, p)


def _fn_replicated(x, src, dst, norm, selfnorm, p):
    h = jax.nn.relu(x @ p["in_W"] + p["in_b"])               # [N, D] replicated
    h = _gcn_layers(h, src, dst, norm, selfnorm, p)
    d = lax.axis_index("i")
    hg = lax.dynamic_slice_in_dim(h, d * NPD, NPD, axis=0)
    return _tf_and_head(hg, p)


def _prep(edge_index):
    src = edge_index[0].astype(np.int32)
    dst = edge_index[1].astype(np.int32)
    deg = np.bincount(dst, minlength=N).astype(np.float32) + 1.0
    dinv = 1.0 / np.sqrt(np.maximum(deg, 1.0))
    norm = (dinv[src] * dinv[dst]).astype(np.float32)
    selfnorm = (dinv * dinv).astype(np.float32)
    return src, dst, norm, selfnorm


_PMAP_CACHE = {}


def _get_pmap(kind):
    f = _PMAP_CACHE.get(kind)
    if f is None:
        devs = jax.devices()
        if len(devs) < M:
            raise RuntimeError("need 8 devices")
        if kind == "ag":
            f = jax.pmap(_fn_allgather, axis_name="i",
                         in_axes=(0, None, None, None, None, None),
                         devices=devs[:M])
        else:
            f = jax.pmap(_fn_replicated, axis_name="i",
                         in_axes=(None, None, None, None, None, None),
                         devices=devs[:M])
        _PMAP_CACHE[kind] = f
    return f


def _run_neuron(x, edge_index, params):
    src, dst, norm, selfnorm = _prep(edge_index)
    p = {k: jnp.asarray(v) for k, v in params.items()}
    xs = np.asarray(x).reshape(M, NPD, F_IN)
    try:
        out = _get_pmap("ag")(xs, jnp.asarray(src), jnp.asarray(dst),
                              jnp.asarray(norm), jnp.asarray(selfnorm), p)
        return np.asarray(out).reshape(B, NCLS)
    except Exception:
        out = _get_pmap("rep")(jnp.asarray(np.asarray(x)), jnp.asarray(src),
                               jnp.asarray(dst), jnp.asarray(norm),
                               jnp.asarray(selfnorm), p)
        return np.asarray(out).reshape(B, NCLS)


def _run_cpu(x, edge_index, batch, params):
    cpu = jax.devices("cpu")[0]
    with jax.default_device(cpu):
        src, dst, norm, selfnorm = _prep(edge_index)
        p = {k: jnp.asarray(np.asarray(v)) for k, v in params.items()}
        h = jax.nn.relu(jnp.asarray(np.asarray(x)) @ p["in_W"] + p["in_b"])
        h = _gcn_layers(h, jnp.asarray(src), jnp.asarray(dst),
                        jnp.asarray(norm), jnp.asarray(selfnorm), p)
        outs = []
        for d in range(M):
            outs.append(_tf_and_head(h[d * NPD:(d + 1) * NPD], p))
        return np.concatenate([np.asarray(o) for o in outs], axis=0)


def kernel(x, edge_index, batch, params):
    try:
        return _run_neuron(x, edge_index, params)
    except Exception:
        return _run_cpu(x, edge_index, batch, params)
